# revision 12
# baseline (speedup 1.0000x reference)
"""Trainium2 Bass kernel for the bidirectional joint updater (GNN message passing).

Math (see reference):
    Tu = relu(user @ W.T + b); Ti = relu(item @ W.T + b)
    S  = user @ item.T                      [N, M]
    E  = exp(S - C) * mask                  (C = global shift; softmax-invariant)
    updated_user = Tu + (E @ item) / rowsum(E)
    updated_item = Ti + (E.T @ user) / colsum(E)

Sharding: users split across 8 cores (2048 rows each), items replicated.
Each core computes E for its user rows once and feeds both directions:
  - user direction fully on-device (row sums are core-local)
  - item direction produces partials (E.T @ user, colsum(E)); host sums
    across cores and finishes the division + Ti add.

Per-core device dataflow (v8). Users processed in two halves of 1024 so the
PSUM accumulators fit; item chunks of 128 in groups of 4:
  S^T piece [128 it, 512 us] via f32r matmul (precision anchor) -> PSUM
  DVE tensor_add of fp8 additive mask {0,-240} in-place in PSUM
  exp(S - 56) on ACT -> E^T tile (bf16), accum_out -> item colsum partials c
  uctx^T += inat_j.T @ E^T           (bf16 matmul, PSUM accumulator [D,1024])
  r_row  += ones.T @ E^T             (bf16 matmul, [1,512] PSUM accumulators)
  E^T tile -> DRAM scratch; DMA-transpose back -> E natural (bf16, SBUF)
  pnum^T(half) += unat_u.T @ E_nat   (bf16, 1-bank PSUM per 512-item group)
Finale per half: r -> [128,8] via tiny scatter DMA, reciprocal; updated_user
= Tu + uctx * (1/r), transposed to natural layout on the PE and DMA'd out.
Host: sum pnum/c over cores+halves, divide, add Ti (numpy).
"""

import sys

if "/opt/trn_rl_repo" not in sys.path:
    sys.path.insert(0, "/opt/trn_rl_repo")

from contextlib import ExitStack

import ml_dtypes
import numpy as np

import concourse.bass as bass
import concourse.tile as tile
from concourse import bacc, mybir
from concourse.mybir import AluOpType as alu
from concourse.mybir import ActivationFunctionType as act

F32 = mybir.dt.float32
F32R = mybir.dt.float32r
BF16 = mybir.dt.bfloat16
F8 = mybir.dt.float8e4
FP8_NP = ml_dtypes.float8_e4m3
BF16_NP = ml_dtypes.bfloat16

C_SHIFT = 56.0  # global logit shift: keeps exp() in fp32 range (S in [-82, 93])

N_FULL, M_FULL, D = 16384, 8192, 128
N_CORES = 8
NS_FULL = N_FULL // N_CORES


def build_kernel(ns=NS_FULL, m=M_FULL):
    """Build the per-core Bass program. Returns the compiled Bacc object."""
    P = 128
    HW = ns // 2              # users per half
    n_uc = HW // P            # user chunks per half
    n_ic = m // P             # item chunks
    G = 4                     # item chunks per group (512-item pnum slices)
    n_g = n_ic // G
    n_s = HW // 512           # S matmul pieces per (chunk, half)
    assert HW % 512 == 0 and m % (G * P) == 0

    nc = bacc.Bacc("TRN2", target_bir_lowering=False, debug=False)

    # ---- DRAM I/O ----
    userT_d = nc.dram_tensor("userT", [D, ns], F32R, kind="ExternalInput")
    unat_d = nc.dram_tensor("unat", [P, 2 * n_uc * D], BF16, kind="ExternalInput")
    itemT_d = nc.dram_tensor("itemT", [D, m], F32R, kind="ExternalInput")
    inat_d = nc.dram_tensor("inat", [P, n_ic * D], BF16, kind="ExternalInput")
    mT_d = nc.dram_tensor("mT", [2 * m, HW], F8, kind="ExternalInput")
    WT_d = nc.dram_tensor("WT", [D, D], F32R, kind="ExternalInput")
    b_d = nc.dram_tensor("bias", [D, 1], F32, kind="ExternalInput")
    ident_d = nc.dram_tensor("ident", [P, P], F32, kind="ExternalInput")

    upd_user_d = nc.dram_tensor("upd_user", [ns, D], F32, kind="ExternalOutput")
    pnumT_d = nc.dram_tensor("pnumT", [2, D, m], F32, kind="ExternalOutput")
    cparts_d = nc.dram_tensor(
        "c_parts", [P, n_ic * 2 * n_s], F32, kind="ExternalOutput"
    )
    edram = nc.dram_tensor("edram", [n_ic, P, ns], BF16)  # E^T scratch
    r_dram = nc.dram_tensor("r_dram", [1, ns], F32)  # r bounce for scatter

    with tile.TileContext(nc) as tc, ExitStack() as ctx:
        const = ctx.enter_context(tc.tile_pool(name="const", bufs=1))
        mpool = ctx.enter_context(tc.tile_pool(name="mpool", bufs=3))
        etp = ctx.enter_context(tc.tile_pool(name="etp", bufs=2 * G + 2))
        enatp = ctx.enter_context(tc.tile_pool(name="enatp", bufs=3))
        pnp = ctx.enter_context(tc.tile_pool(name="pnp", bufs=2))
        finp = ctx.enter_context(tc.tile_pool(name="finp", bufs=3))
        ps_s = ctx.enter_context(tc.tile_pool(name="ps_s", bufs=3, space="PSUM"))
        ps_u = ctx.enter_context(tc.tile_pool(name="ps_u", bufs=1, space="PSUM"))
        ps_r = ctx.enter_context(tc.tile_pool(name="ps_r", bufs=2, space="PSUM"))
        ps_pn = ctx.enter_context(tc.tile_pool(name="ps_pn", bufs=1, space="PSUM"))

        # ---- preload constants ----
        userT = const.tile([D, ns], F32R, name="userT_s")
        nc.sync.dma_start(userT[:], userT_d[:])
        unat = const.tile([P, 2 * n_uc * D], BF16, name="unat_s")
        nc.sync.dma_start(unat[:], unat_d[:])
        itemT = const.tile([D, m], F32R, name="itemT_s")
        nc.sync.dma_start(itemT[:], itemT_d[:])
        inat = const.tile([P, n_ic * D], BF16, name="inat_s")
        nc.sync.dma_start(inat[:], inat_d[:])
        WT = const.tile([D, D], F32R, name="WT_s")
        nc.sync.dma_start(WT[:], WT_d[:])
        bcol = const.tile([D, 1], F32, name="b_s")
        nc.sync.dma_start(bcol[:], b_d[:])
        ident = const.tile([P, P], F32, name="ident_s")
        nc.sync.dma_start(ident[:], ident_d[:])

        c_parts = const.tile([P, n_ic * 2 * n_s], F32, name="c_parts")
        TuT = const.tile([D, ns], F32, name="TuT")
        uctxT_sb = const.tile([D, HW], F32, name="uctxT_sb")
        r_row = const.tile([1, ns], F32, name="r_row")
        r_nat = const.tile([P, 2 * n_uc], F32, name="r_nat")
        recip = const.tile([P, 2 * n_uc], F32, name="recip")
        negC = const.tile([P, 1], F32, name="negC")
        nc.vector.memset(negC[:], -C_SHIFT)
        ones = const.tile([P, 1], BF16, name="ones_s")
        nc.vector.memset(ones[:], 1.0)

        # ---- Tu^T = relu(W @ user^T + b), [D, ns] ----
        for h in range(2 * n_s):
            tp = ps_s.tile([P, 512], F32, tag="s", name="tu_ps")
            nc.tensor.matmul(tp[:], WT[:], userT[:, bass.ts(h, 512)])
            nc.scalar.activation(
                TuT[:, bass.ts(h, 512)], tp[:], act.Relu, bias=bcol[:, 0:1]
            )

        for half in range(2):
            uctx_ps = ps_u.tile([D, HW], F32, tag="u", name="uctx_ps")
            r_ps = [
                ps_r.tile([P, 512], F32, tag="r", name="r_ps") for _ in range(n_s)
            ]
            for g in range(n_g):
                et_tiles = []
                for jj in range(G):
                    j = g * G + jj
                    mt = mpool.tile([P, HW], F8, name="mt")
                    nc.sync.dma_start(
                        mt[:], mT_d[half * m + j * P : half * m + (j + 1) * P, :]
                    )
                    et = etp.tile([P, HW], BF16, tag="et", name="et")
                    et_tiles.append(et)
                    for h in range(n_s):
                        sp = ps_s.tile([P, 512], F32, tag="s", name="s_ps")
                        nc.tensor.matmul(
                            sp[:],
                            itemT[:, j * P : (j + 1) * P],
                            userT[:, half * HW + h * 512 : half * HW + (h + 1) * 512],
                        )
                        nc.vector.tensor_add(sp[:], sp[:], mt[:, bass.ts(h, 512)])
                        slot = j * 2 * n_s + half * n_s + h
                        nc.scalar.activation(
                            et[:, bass.ts(h, 512)],
                            sp[:],
                            act.Exp,
                            bias=negC[:, 0:1],
                            accum_out=c_parts[:, slot : slot + 1],
                        )
                        # uctx^T and r accumulation off the fresh E^T piece
                        nc.tensor.matmul(
                            uctx_ps[:, bass.ts(h, 512)],
                            inat[:, j * D : (j + 1) * D],
                            et[:, bass.ts(h, 512)],
                            start=(j == 0),
                            stop=(j == n_ic - 1),
                        )
                        nc.tensor.matmul(
                            r_ps[h][0:1, :],
                            ones[:],
                            et[:, bass.ts(h, 512)],
                            start=(j == 0),
                            stop=(j == n_ic - 1),
                        )
                    # E^T -> DRAM scratch for the transposed reload
                    nc.sync.dma_start(
                        edram[j, :, half * HW : (half + 1) * HW], et[:]
                    )

                # transposed reload + pnum accumulation
                pn_ps = ps_pn.tile([D, G * P], F32, tag="pn", name="pn_ps")
                for u in range(n_uc):
                    enat = enatp.tile([P, G * P], BF16, name="enat")
                    for jj in range(G):
                        j = g * G + jj
                        nc.sync.dma_start_transpose(
                            enat[:, jj * P : (jj + 1) * P],
                            edram[j, :, half * HW + u * P : half * HW + (u + 1) * P],
                        )
                    nc.tensor.matmul(
                        pn_ps[:],
                        unat[:, (half * n_uc + u) * D : (half * n_uc + u + 1) * D],
                        enat[:],
                        start=(u == 0),
                        stop=(u == n_uc - 1),
                    )
                pn_sb = pnp.tile([D, G * P], F32, name="pn_sb")
                nc.vector.tensor_copy(pn_sb[:], pn_ps[:])
                nc.sync.dma_start(
                    pnumT_d[half, :, g * G * P : (g + 1) * G * P], pn_sb[:]
                )

            # ---- per-half finale ----
            for h in range(n_s):
                nc.vector.tensor_copy(
                    r_row[0:1, half * HW + h * 512 : half * HW + (h + 1) * 512],
                    r_ps[h][0:1, :],
                )
            for k in range(n_s):
                nc.scalar.copy(
                    uctxT_sb[:, bass.ts(k, 512)], uctx_ps[:, bass.ts(k, 512)]
                )
            # r [1, HW] -> [128, n_uc] partition-major scatter via DRAM bounce
            nc.sync.dma_start(
                r_dram[0:1, half * HW : (half + 1) * HW],
                r_row[0:1, half * HW : (half + 1) * HW],
            )
            nc.sync.dma_start(
                r_nat[:, half * n_uc : (half + 1) * n_uc],
                r_dram[0:1, half * HW : (half + 1) * HW].rearrange(
                    "a (c p) -> (a p) c", p=P
                ),
            )
            nc.vector.reciprocal(
                recip[:, half * n_uc : (half + 1) * n_uc],
                r_nat[:, half * n_uc : (half + 1) * n_uc],
            )
            for u in range(n_uc):
                ug = half * n_uc + u
                fp = ps_s.tile([P, 512], F32, tag="s", name="fin_ps")
                nc.tensor.transpose(
                    fp[:, 0:P], uctxT_sb[:, u * P : (u + 1) * P], ident[:]
                )
                nc.tensor.transpose(
                    fp[:, P : 2 * P], TuT[:, ug * P : (ug + 1) * P], ident[:]
                )
                scaled = finp.tile([P, P], F32, name="scaled")
                nc.vector.tensor_scalar(
                    scaled[:], fp[:, 0:P], recip[:, ug : ug + 1], None, op0=alu.mult
                )
                upd = finp.tile([P, P], F32, name="upd")
                nc.vector.tensor_add(upd[:], fp[:, P : 2 * P], scaled[:])
                nc.sync.dma_start(upd_user_d[ug * P : (ug + 1) * P, :], upd[:])

        nc.sync.dma_start(cparts_d[:], c_parts[:])

    nc.compile()
    return nc


def _prep_core_inputs(user_emb, item_emb, mask_bool, W, b, ns, m):
    """Host-side layout prep. Shared tensors computed once by the caller."""
    P = 128
    n_ic = m // P
    HW = ns // 2
    itemT = np.ascontiguousarray(item_emb.T)
    inat = np.ascontiguousarray(
        item_emb.astype(BF16_NP).reshape(n_ic, P, D).transpose(1, 0, 2).reshape(P, n_ic * D)
    )
    shared = {
        "itemT": itemT,
        "inat": inat,
        "WT": np.ascontiguousarray(W.T),
        "bias": np.ascontiguousarray(b.reshape(D, 1)),
        "ident": np.eye(P, dtype=np.float32),
    }
    in_maps = []
    n_uct = ns // P
    for c in range(user_emb.shape[0] // ns):
        sl = user_emb[c * ns : (c + 1) * ns]
        im = dict(shared)
        im["userT"] = np.ascontiguousarray(sl.T)
        im["unat"] = np.ascontiguousarray(
            sl.astype(BF16_NP).reshape(n_uct, P, D).transpose(1, 0, 2).reshape(P, n_uct * D)
        )
        mTb = np.where(
            np.ascontiguousarray(mask_bool[c * ns : (c + 1) * ns].T), 0.0, -240.0
        ).astype(FP8_NP)
        im["mT"] = np.ascontiguousarray(
            np.concatenate([mTb[:, :HW], mTb[:, HW:]], axis=0)
        )
        in_maps.append(im)
    return in_maps


def _combine(results, user_emb, item_emb, W, b, ns, m):
    P = 128
    n_cs = ns // 512  # c slots per item chunk (2 * n_s)
    n_ic = m // P
    upd_user = np.concatenate([r["upd_user"] for r in results], axis=0)
    num = np.sum(
        [r["pnumT"].sum(axis=0) for r in results], axis=0, dtype=np.float64
    )
    cp = np.sum([r["c_parts"] for r in results], axis=0, dtype=np.float64)
    den = cp.reshape(P, n_ic, n_cs).sum(-1).T.reshape(m)  # den[j*128+p]
    Ti = np.maximum(item_emb @ W.T + b, 0.0)
    upd_item = (Ti + (num / den[None, :]).T).astype(np.float32)
    return upd_user.astype(np.float32), upd_item


_NC_CACHE = {}


def kernel(user_emb, item_emb, adj_matrix, W, b, _trace=False):
    user_emb = np.ascontiguousarray(np.asarray(user_emb, dtype=np.float32))
    item_emb = np.ascontiguousarray(np.asarray(item_emb, dtype=np.float32))
    W = np.asarray(W, dtype=np.float32)
    b = np.asarray(b, dtype=np.float32)
    mask_bool = np.asarray(adj_matrix) > 0

    ns, m = NS_FULL, M_FULL
    if "nc" not in _NC_CACHE:
        _NC_CACHE["nc"] = build_kernel(ns, m)
    nc = _NC_CACHE["nc"]

    in_maps = _prep_core_inputs(user_emb, item_emb, mask_bool, W, b, ns, m)

    from concourse.bass_utils import run_bass_kernel_spmd

    res = run_bass_kernel_spmd(
        nc, in_maps, core_ids=list(range(N_CORES)), trace=_trace
    )
    _NC_CACHE["last_results"] = res
    out = _combine(res.results, user_emb, item_emb, W, b, ns, m)
    return out


# revision 13
# speedup vs baseline: 5.2194x; 5.2194x over previous
"""Trainium2 Bass kernel for the bidirectional joint updater (GNN message passing).

Math (see reference):
    Tu = relu(user @ W.T + b); Ti = relu(item @ W.T + b)
    S  = user @ item.T                      [N, M]
    E  = exp(S - C) * mask                  (C = global shift; softmax-invariant)
    updated_user = Tu + (E @ item) / rowsum(E)
    updated_item = Ti + (E.T @ user) / colsum(E)

Sharding: users split across 8 cores (2048 rows each), items replicated.
Each core computes E for its user rows once and feeds both directions:
  - user direction fully on-device (row sums are core-local)
  - item direction produces partials (E.T @ user, colsum(E)); host sums
    across cores and finishes the division + Ti add (numpy).

Per-core device dataflow (v9). Users processed in two halves of 1024 (PSUM
budget); item chunks of 128 in groups of 4:
  S^T piece [128 it, 512 us] via f32r matmul (precision anchor) -> PSUM
  DVE tensor_add of fp8 additive mask {0,-240} in-place in PSUM
  exp(S - 56) on ACT -> E^T tile (bf16), accum_out -> item colsum partials c
  uctx^T += inat_j.T @ E^T            (bf16, PSUM accumulator [D, 1024])
  PE-transpose E^T -> E natural (bf16, PSUM, double-buffered bank)
  copy PSUM->SBUF with accum_out      (row-sum partials r, ACT/DVE split)
  pnum^T(half) += unat_u.T @ E_nat    (bf16, 1-bank PSUM per 512-item group)
Finale per half: reduce r partials, reciprocal; updated_user = Tu + uctx/r,
transposed to natural layout on the PE and DMA'd out.
"""

import sys

if "/opt/trn_rl_repo" not in sys.path:
    sys.path.insert(0, "/opt/trn_rl_repo")

from contextlib import ExitStack

import ml_dtypes
import numpy as np

import concourse.bass as bass
import concourse.tile as tile
from concourse import bacc, mybir
from concourse.mybir import AluOpType as alu
from concourse.mybir import ActivationFunctionType as act

F32 = mybir.dt.float32
F32R = mybir.dt.float32r
BF16 = mybir.dt.bfloat16
F8 = mybir.dt.float8e4
FP8_NP = ml_dtypes.float8_e4m3
BF16_NP = ml_dtypes.bfloat16

C_SHIFT = 56.0  # global logit shift: keeps exp() in fp32 range (S in [-82, 93])

N_FULL, M_FULL, D = 16384, 8192, 128
N_CORES = 8
NS_FULL = N_FULL // N_CORES


def build_kernel(ns=NS_FULL, m=M_FULL, act_copy_mod=2):
    """Build the per-core Bass program. Returns the compiled Bacc object."""
    P = 128
    HW = ns // 2              # users per half
    n_uc = HW // P            # user chunks per half
    n_ic = m // P             # item chunks
    G = 4                     # item chunks per group (512-item pnum slices)
    n_g = n_ic // G
    n_s = HW // 512           # S matmul pieces per (chunk, half)
    assert HW % 512 == 0 and m % (G * P) == 0

    nc = bacc.Bacc("TRN2", target_bir_lowering=False, debug=False)

    # ---- DRAM I/O ----
    userT_d = nc.dram_tensor("userT", [D, ns], F32R, kind="ExternalInput")
    unat_d = nc.dram_tensor("unat", [P, 2 * n_uc * D], BF16, kind="ExternalInput")
    itemT_d = nc.dram_tensor("itemT", [D, m], F32R, kind="ExternalInput")
    inat_d = nc.dram_tensor("inat", [P, n_ic * D], BF16, kind="ExternalInput")
    mT_d = nc.dram_tensor("mT", [2 * m, HW], F8, kind="ExternalInput")
    WT_d = nc.dram_tensor("WT", [D, D], F32R, kind="ExternalInput")
    b_d = nc.dram_tensor("bias", [D, 1], F32, kind="ExternalInput")
    ident_d = nc.dram_tensor("ident", [P, P], F32, kind="ExternalInput")
    identb_d = nc.dram_tensor("identb", [P, P], BF16, kind="ExternalInput")

    upd_user_d = nc.dram_tensor("upd_user", [ns, D], F32, kind="ExternalOutput")
    pnumT_d = nc.dram_tensor("pnumT", [2, D, m], F32, kind="ExternalOutput")
    cparts_d = nc.dram_tensor(
        "c_parts", [P, n_ic * 2 * n_s], F32, kind="ExternalOutput"
    )

    with tile.TileContext(nc) as tc, ExitStack() as ctx:
        const = ctx.enter_context(tc.tile_pool(name="const", bufs=1))
        mpool = ctx.enter_context(tc.tile_pool(name="mpool", bufs=3))
        etp = ctx.enter_context(tc.tile_pool(name="etp", bufs=2 * G + 2))
        enatp = ctx.enter_context(tc.tile_pool(name="enatp", bufs=3))
        pnp = ctx.enter_context(tc.tile_pool(name="pnp", bufs=2))
        finp = ctx.enter_context(tc.tile_pool(name="finp", bufs=3))
        ps_s = ctx.enter_context(tc.tile_pool(name="ps_s", bufs=3, space="PSUM"))
        ps_u = ctx.enter_context(tc.tile_pool(name="ps_u", bufs=1, space="PSUM"))
        ps_tr = ctx.enter_context(tc.tile_pool(name="ps_tr", bufs=2, space="PSUM"))
        ps_pn = ctx.enter_context(tc.tile_pool(name="ps_pn", bufs=1, space="PSUM"))

        # ---- preload constants ----
        userT = const.tile([D, ns], F32R, name="userT_s")
        nc.sync.dma_start(userT[:], userT_d[:])
        unat = const.tile([P, 2 * n_uc * D], BF16, name="unat_s")
        nc.sync.dma_start(unat[:], unat_d[:])
        itemT = const.tile([D, m], F32R, name="itemT_s")
        nc.sync.dma_start(itemT[:], itemT_d[:])
        inat = const.tile([P, n_ic * D], BF16, name="inat_s")
        nc.sync.dma_start(inat[:], inat_d[:])
        WT = const.tile([D, D], F32R, name="WT_s")
        nc.sync.dma_start(WT[:], WT_d[:])
        bcol = const.tile([D, 1], F32, name="b_s")
        nc.sync.dma_start(bcol[:], b_d[:])
        ident = const.tile([P, P], F32, name="ident_s")
        nc.sync.dma_start(ident[:], ident_d[:])
        identb = const.tile([P, P], BF16, name="identb_s")
        nc.sync.dma_start(identb[:], identb_d[:])

        c_parts = const.tile([P, n_ic * 2 * n_s], F32, name="c_parts")
        r_parts = const.tile([P, 2 * n_uc * n_g], F32, name="r_parts")
        TuT = const.tile([D, ns], F32, name="TuT")
        uctxT_sb = const.tile([D, HW], F32, name="uctxT_sb")
        rsum = const.tile([P, 2 * n_uc], F32, name="rsum")
        recip = const.tile([P, 2 * n_uc], F32, name="recip")
        negC = const.tile([P, 1], F32, name="negC")
        nc.vector.memset(negC[:], -C_SHIFT)

        # ---- Tu^T = relu(W @ user^T + b), [D, ns] ----
        for h in range(2 * n_s):
            tp = ps_s.tile([P, 512], F32, tag="s", name="tu_ps")
            nc.tensor.matmul(tp[:], WT[:], userT[:, bass.ts(h, 512)])
            nc.scalar.activation(
                TuT[:, bass.ts(h, 512)], tp[:], act.Relu, bias=bcol[:, 0:1]
            )

        copy_idx = 0
        for half in range(2):
            uctx_ps = ps_u.tile([D, HW], F32, tag="u", name="uctx_ps")
            for g in range(n_g):
                et_tiles = []
                for jj in range(G):
                    j = g * G + jj
                    mt = mpool.tile([P, HW], F8, name="mt")
                    nc.sync.dma_start(
                        mt[:], mT_d[half * m + j * P : half * m + (j + 1) * P, :]
                    )
                    et = etp.tile([P, HW], BF16, tag="et", name="et")
                    et_tiles.append(et)
                    for h in range(n_s):
                        sp = ps_s.tile([P, 512], F32, tag="s", name="s_ps")
                        nc.tensor.matmul(
                            sp[:],
                            itemT[:, j * P : (j + 1) * P],
                            userT[:, half * HW + h * 512 : half * HW + (h + 1) * 512],
                        )
                        nc.vector.tensor_add(sp[:], sp[:], mt[:, bass.ts(h, 512)])
                        slot = j * 2 * n_s + half * n_s + h
                        nc.scalar.activation(
                            et[:, bass.ts(h, 512)],
                            sp[:],
                            act.Exp,
                            bias=negC[:, 0:1],
                            accum_out=c_parts[:, slot : slot + 1],
                        )
                        nc.tensor.matmul(
                            uctx_ps[:, bass.ts(h, 512)],
                            inat[:, j * D : (j + 1) * D],
                            et[:, bass.ts(h, 512)],
                            start=(j == 0),
                            stop=(j == n_ic - 1),
                        )

                # transpose E^T -> E natural; copy w/ r-accum; pnum
                pn_ps = ps_pn.tile([D, G * P], F32, tag="pn", name="pn_ps")
                for u in range(n_uc):
                    trp = ps_tr.tile([P, G * P], BF16, tag="tr", name="tr_ps")
                    for jj in range(G):
                        nc.tensor.transpose(
                            trp[:, jj * P : (jj + 1) * P],
                            et_tiles[jj][:, u * P : (u + 1) * P],
                            identb[:],
                        )
                    enat = enatp.tile([P, G * P], BF16, name="enat")
                    ug = half * n_uc + u
                    rslot = r_parts[:, ug * n_g + g : ug * n_g + g + 1]
                    if copy_idx % act_copy_mod == 0:
                        nc.scalar.activation(
                            enat[:], trp[:], act.Copy, accum_out=rslot
                        )
                    else:
                        nc.vector.tensor_scalar(
                            enat[:], trp[:], 1.0, None,
                            op0=alu.mult, op1=alu.add, accum_out=rslot,
                        )
                    copy_idx += 1
                    nc.tensor.matmul(
                        pn_ps[:],
                        unat[:, ug * D : (ug + 1) * D],
                        enat[:],
                        start=(u == 0),
                        stop=(u == n_uc - 1),
                    )
                pn_sb = pnp.tile([D, G * P], F32, name="pn_sb")
                nc.vector.tensor_copy(pn_sb[:], pn_ps[:])
                nc.sync.dma_start(
                    pnumT_d[half, :, g * G * P : (g + 1) * G * P], pn_sb[:]
                )

            # ---- per-half finale ----
            for k in range(n_s):
                nc.scalar.copy(
                    uctxT_sb[:, bass.ts(k, 512)], uctx_ps[:, bass.ts(k, 512)]
                )
            nc.vector.tensor_reduce(
                rsum[:, half * n_uc : (half + 1) * n_uc],
                r_parts[:, half * n_uc * n_g : (half + 1) * n_uc * n_g].rearrange(
                    "p (u g) -> p u g", g=n_g
                ),
                axis=mybir.AxisListType.X,
                op=alu.add,
            )
            nc.vector.reciprocal(
                recip[:, half * n_uc : (half + 1) * n_uc],
                rsum[:, half * n_uc : (half + 1) * n_uc],
            )
            for u in range(n_uc):
                ug = half * n_uc + u
                fp = ps_s.tile([P, 512], F32, tag="s", name="fin_ps")
                nc.tensor.transpose(
                    fp[:, 0:P], uctxT_sb[:, u * P : (u + 1) * P], ident[:]
                )
                nc.tensor.transpose(
                    fp[:, P : 2 * P], TuT[:, ug * P : (ug + 1) * P], ident[:]
                )
                scaled = finp.tile([P, P], F32, name="scaled")
                nc.vector.tensor_scalar(
                    scaled[:], fp[:, 0:P], recip[:, ug : ug + 1], None, op0=alu.mult
                )
                upd = finp.tile([P, P], F32, name="upd")
                nc.vector.tensor_add(upd[:], fp[:, P : 2 * P], scaled[:])
                nc.sync.dma_start(upd_user_d[ug * P : (ug + 1) * P, :], upd[:])

        nc.sync.dma_start(cparts_d[:], c_parts[:])

    nc.compile()
    return nc


def _prep_core_inputs(user_emb, item_emb, mask_bool, W, b, ns, m):
    """Host-side layout prep. Shared tensors computed once by the caller."""
    P = 128
    n_ic = m // P
    HW = ns // 2
    itemT = np.ascontiguousarray(item_emb.T)
    inat = np.ascontiguousarray(
        item_emb.astype(BF16_NP).reshape(n_ic, P, D).transpose(1, 0, 2).reshape(P, n_ic * D)
    )
    shared = {
        "itemT": itemT,
        "inat": inat,
        "WT": np.ascontiguousarray(W.T),
        "bias": np.ascontiguousarray(b.reshape(D, 1)),
        "ident": np.eye(P, dtype=np.float32),
        "identb": np.eye(P, dtype=np.float32).astype(BF16_NP),
    }
    in_maps = []
    n_uct = ns // P
    for c in range(user_emb.shape[0] // ns):
        sl = user_emb[c * ns : (c + 1) * ns]
        im = dict(shared)
        im["userT"] = np.ascontiguousarray(sl.T)
        im["unat"] = np.ascontiguousarray(
            sl.astype(BF16_NP).reshape(n_uct, P, D).transpose(1, 0, 2).reshape(P, n_uct * D)
        )
        mTb = np.where(
            np.ascontiguousarray(mask_bool[c * ns : (c + 1) * ns].T), 0.0, -240.0
        ).astype(FP8_NP)
        im["mT"] = np.ascontiguousarray(
            np.concatenate([mTb[:, :HW], mTb[:, HW:]], axis=0)
        )
        in_maps.append(im)
    return in_maps


def _combine(results, user_emb, item_emb, W, b, ns, m):
    P = 128
    n_cs = ns // 512  # c slots per item chunk (2 * n_s)
    n_ic = m // P
    upd_user = np.concatenate([r["upd_user"] for r in results], axis=0)
    num = np.sum(
        [r["pnumT"].sum(axis=0) for r in results], axis=0, dtype=np.float64
    )
    cp = np.sum([r["c_parts"] for r in results], axis=0, dtype=np.float64)
    den = cp.reshape(P, n_ic, n_cs).sum(-1).T.reshape(m)  # den[j*128+p]
    Ti = np.maximum(item_emb @ W.T + b, 0.0)
    upd_item = (Ti + (num / den[None, :]).T).astype(np.float32)
    return upd_user.astype(np.float32), upd_item


_NC_CACHE = {}


def kernel(user_emb, item_emb, adj_matrix, W, b, _trace=False):
    user_emb = np.ascontiguousarray(np.asarray(user_emb, dtype=np.float32))
    item_emb = np.ascontiguousarray(np.asarray(item_emb, dtype=np.float32))
    W = np.asarray(W, dtype=np.float32)
    b = np.asarray(b, dtype=np.float32)
    mask_bool = np.asarray(adj_matrix) > 0

    ns, m = NS_FULL, M_FULL
    if "nc" not in _NC_CACHE:
        _NC_CACHE["nc"] = build_kernel(ns, m)
    nc = _NC_CACHE["nc"]

    in_maps = _prep_core_inputs(user_emb, item_emb, mask_bool, W, b, ns, m)

    from concourse.bass_utils import run_bass_kernel_spmd

    res = run_bass_kernel_spmd(
        nc, in_maps, core_ids=list(range(N_CORES)), trace=_trace
    )
    _NC_CACHE["last_results"] = res
    out = _combine(res.results, user_emb, item_emb, W, b, ns, m)
    return out
